# revision 26
# baseline (speedup 1.0000x reference)
"""Segment-mean GNN message passing (scatter-mean) on 8 TRN2 NeuronCores.

out[d] = mean over edges e with col[e]==d of x[row[e]]   (empty segments -> 0)

Design (1D destination partition per the sharding hint):
- Dests sharded across 8 cores (6250 each); edges partitioned by dest on
  host, which also gathers x[row] per edge slot (the halo exchange) into a
  bf16 tile stream. Full floor(d/3) triple groups live in 128-group tiles
  sharing one one-hot scatter matrix; each chunk-triplet's mod-3 leftover
  edges share ONE remainder tile whose three slot-columns carry three
  distinct one-hots into three different psum accumulators. Dest->chunk
  assignment is bin-packed (LPT + swap repair + mod pre-promotion) against
  a shared 5/6-tile profile, hitting the exact capacity lower bound
  (263 tiles/core, <1% padding vs raw edges).
- TensorE scatters each slot column via one-hot matmuls (cost is
  output-width-bound, so contraction passes are cheap); VectorE builds the
  one-hot matrices (is_equal vs an on-device iota) far ahead of the input
  stream; ScalarE+VectorE divide by degree writing bf16.
- One DMA per triplet (per-chunk for the last two); output DMAs write a
  packed partition-major DRAM layout (512B descriptors) and are
  release-gated so their transfers queue behind all input transfers; the
  final group goes out through a prepared SWDGE kv_writeback fired by a
  ~60ns trigger. Host converts bf16->f32 and inverts the permutation.
The DMA engine stream is fully packed: total time = first-DMA issue
latency + bytes/360GBps + completion-semaphore close-out.
"""

import sys

for _p in ("/opt/trn_rl_repo",):
    if _p not in sys.path:
        sys.path.insert(0, _p)

import numpy as np
import ml_dtypes

N_NODES = 50000
D_FEAT = 64
N_EDGES = 800000
NCORES = 8
SPAN = N_NODES // NCORES  # 6250 dests per core
P = 128
NCHUNK = (SPAN + P - 1) // P  # 49 (last chunk has 106 dests)
G = 3
NTRIP = 16  # chunks 0..47 in triplets; chunk 48 (partial) standalone
PS_BUFS = 6  # psum accumulators: triplet t uses 3*(t%2)+{0,1,2}
OHB = 24  # one-hot matrix ring depth
OUT_BOUNDS = [0, 8, 16, 24, 32, 40, 45, 49]
OUT_RELEASE = [33, 36, 39, 42, 45, 45]


def _pack_bins_v2(qd_in, md_in, ncap640, seed=0, mslack=120):
    """Pack SPAN dests into 48 main bins (128 dests each) + a partial bin
    (106): ncap640 main bins capped at 640 quads (5 tiles), rest at 768;
    every main bin's mod-sum <= 128 (remainder-column capacity), repaired
    by quad-equal swaps or promotion to ceil groups. The partial bin uses
    ceil(d/3) groups. Returns (newpos, qd, md, mainsums) or None."""
    import heapq

    nd = len(qd_in)
    qd = qd_in.copy()
    md = md_in.copy()
    npart = SPAN - (NCHUNK - 1) * P  # 106
    nmain = NCHUNK - 1  # 48
    qceil = qd + (md > 0)
    asc = np.argsort(qceil, kind="stable")
    part = asc[:npart]
    inpart = np.zeros(nd, bool)
    inpart[part] = True
    qd[part] = qceil[part]
    md[part] = 0

    main = np.where(~inpart)[0]
    if seed:
        rng = np.random.RandomState(seed)
        jitter = rng.rand(len(main))
        main = main[np.lexsort((jitter, -qd[main]))]
    else:
        main = main[np.argsort(-qd[main], kind="stable")]
    caps = np.array([640] * ncap640 + [768] * (nmain - ncap640), np.int64)
    counts = np.zeros(nmain, np.int64)
    sums = np.zeros(nmain, np.int64)
    msums = np.zeros(nmain, np.int64)
    heap = [(-int(caps[b]), b) for b in range(nmain)]
    heapq.heapify(heap)
    members = [[] for _ in range(nmain)]
    for d in main:
        while True:
            _, b = heapq.heappop(heap)
            if counts[b] < P:
                break
        members[b].append(int(d))
        counts[b] += 1
        sums[b] += qd[d]
        msums[b] += md[d]
        if counts[b] < P:
            heapq.heappush(heap, (-(int(caps[b]) - int(sums[b])), b))

    def swap(b, b2, da, db):
        members[b].remove(da)
        members[b2].remove(db)
        members[b].append(db)
        members[b2].append(da)
        sums[b] += qd[db] - qd[da]
        sums[b2] += qd[da] - qd[db]
        msums[b] += md[db] - md[da]
        msums[b2] += md[da] - md[db]

    # repair quad-cap violations
    for _ in range(4000):
        over = np.where(sums > caps)[0]
        if len(over) == 0:
            break
        b = int(over[np.argmax(sums[over] - caps[over])])
        under = np.where(caps - sums > 0)[0]
        done = False
        for b2 in under[np.argsort(-(caps[under] - sums[under]))][:16]:
            b2 = int(b2)
            for da in sorted(members[b], key=lambda d: -qd[d])[:48]:
                for db in sorted(members[b2], key=lambda d: qd[d])[:48]:
                    dq = int(qd[da] - qd[db])
                    if dq > 0 and sums[b2] + dq <= caps[b2]:
                        swap(b, b2, da, db)
                        done = True
                        break
                if done:
                    break
            if done:
                break
        if not done:
            return None
    # repair mod-cap violations (msum <= 128)
    for _ in range(4000):
        over = np.where(msums > P)[0]
        if len(over) == 0:
            break
        b = int(over[0])
        done = False
        under = np.where(msums < P)[0]
        cand_a = sorted([d for d in members[b] if md[d] > 0], key=lambda d: -md[d])[
            :32
        ]
        for b2 in under[np.argsort(msums[under])][:24]:
            b2 = int(b2)
            lowm = {}
            for d in members[b2]:
                q = int(qd[d])
                if q not in lowm or md[d] < md[lowm[q]]:
                    lowm[q] = d
            for da in cand_a:
                db = lowm.get(int(qd[da]))
                if (
                    db is not None
                    and md[db] < md[da]
                    and msums[b2] + md[da] - md[db] <= P
                ):
                    swap(b, b2, da, db)
                    done = True
                    break
            if done:
                break
        if not done:
            # promote a mod!=0 dest to ceil groups if quad room allows
            for da in cand_a:
                if sums[b] + 1 <= caps[b]:
                    sums[b] += 1
                    msums[b] -= md[da]
                    qd[da] += 1
                    md[da] = 0
                    done = True
                    break
            if not done:
                return None
    newpos = np.empty(nd, np.int64)
    for b in range(nmain):
        assert len(members[b]) == P
        newpos[np.array(members[b], np.int64)] = b * P + np.arange(P)
    newpos[part] = nmain * P + np.arange(npart)
    return newpos, qd, md, sums


def _preprocess(x, edge_index):
    x = np.ascontiguousarray(x, dtype=np.float32)
    row = edge_index[0].astype(np.int64)
    col = edge_index[1].astype(np.int64)

    deg = np.bincount(col, minlength=N_NODES).astype(np.int64)
    recip_full = (1.0 / np.maximum(deg, 1.0)).astype(np.float32)

    core = col // SPAN
    lcol = col - core * SPAN

    q_all = deg // G
    m_all = deg - q_all * G

    packs = []
    for ci in range(NCORES):
        qd = q_all[ci * SPAN : (ci + 1) * SPAN]
        md = m_all[ci * SPAN : (ci + 1) * SPAN]
        got = None
        for ncap in (46, 45, 44, 43, 42, 41, 40, 39, 38, 37, 36, 32, 24, 0):
            for seed in (0, 1, 2, 3, 4):
                got = _pack_bins_v2(qd, md, ncap, seed)
                if got is not None:
                    break
            if got is not None:
                break
        assert got is not None, "packing failed"
        packs.append(got)

    binq = np.stack([p[3] for p in packs])  # [cores, 48]
    partq = np.zeros(NCORES, np.int64)
    for ci in range(NCORES):
        newpos, qd2, md2, sums = packs[ci]
        sel = newpos >= (NCHUNK - 1) * P
        partq[ci] = qd2[sel].sum()
    TT = np.maximum(1, -(-binq.max(axis=0) // P)).astype(np.int64)  # [48]
    TP = int(max(1, -(-int(partq.max()) // P)))

    # global tile order: per triplet [A tiles | B | C | remainder], then
    # partial-chunk tiles
    tile_kind = []  # per tile: ("tri", chunk) or ("rem", triplet)
    chunk_tiles = [[] for _ in range(NCHUNK)]
    rem_tile = [0] * NTRIP
    col_of_tile = []
    ncol = 0
    for t in range(NTRIP):
        for g in range(3):
            c = 3 * t + g
            for _k in range(int(TT[c])):
                chunk_tiles[c].append(len(tile_kind))
                tile_kind.append(("tri", c))
                col_of_tile.append(ncol)
                ncol += 1
        rem_tile[t] = len(tile_kind)
        tile_kind.append(("rem", t))
        col_of_tile.append(ncol)
        ncol += 3
    for _k in range(TP):
        chunk_tiles[NCHUNK - 1].append(len(tile_kind))
        tile_kind.append(("tri", NCHUNK - 1))
        col_of_tile.append(ncol)
        ncol += 1
    ttot = len(tile_kind)

    mat_of_tile = np.zeros(ttot + 1, np.int64)
    for ti in range(ttot):
        mat_of_tile[ti + 1] = mat_of_tile[ti] + (
            3 if tile_kind[ti][0] == "rem" else 1
        )
    tile_of_mat = np.zeros(int(mat_of_tile[ttot]), np.int64)
    for ti in range(ttot):
        tile_of_mat[mat_of_tile[ti] : mat_of_tile[ti + 1]] = ti

    # DMA segments: one per triplet, except the last two triplets stream
    # per-chunk (plus their remainder tiles) so the PE/act tail pipelines
    # at fine grain; partial chunk last
    segs = []
    for t in range(NTRIP - 2):
        a = chunk_tiles[3 * t][0]
        b = rem_tile[t] + 1
        segs.append((a, b))
    for t in (NTRIP - 2, NTRIP - 1):
        for g in range(3):
            c = 3 * t + g
            segs.append((chunk_tiles[c][0], chunk_tiles[c][-1] + 1))
        segs.append((rem_tile[t], rem_tile[t] + 1))
    segs.append((rem_tile[NTRIP - 1] + 1, ttot))
    seg_of_tile = np.zeros(ttot, np.int64)
    for s, (a, b) in enumerate(segs):
        seg_of_tile[a:b] = s

    cfg = dict(
        TT=TT,
        TP=TP,
        ttot=ttot,
        ncol=ncol,
        tile_kind=tile_kind,
        chunk_tiles=chunk_tiles,
        rem_tile=rem_tile,
        col_of_tile=col_of_tile,
        mat_of_tile=mat_of_tile,
        tile_of_mat=tile_of_mat,
        segs=segs,
        seg_of_tile=seg_of_tile,
        pos=[p[0] for p in packs],
    )

    tmax = int(TT.max())
    tiles_arr = np.zeros((NCHUNK, max(tmax, TP)), np.int64)
    for c in range(NCHUNK):
        for k, ti in enumerate(chunk_tiles[c]):
            tiles_arr[c, k] = ti
    col_arr = np.array(col_of_tile, np.int64)

    in_maps = []
    for ci in range(NCORES):
        newpos, qd2, md2, sums = packs[ci]
        m = core == ci
        r_i = row[m]
        pe_i = newpos[lcol[m]]
        ch_i = pe_i // P
        d_i = pe_i - ch_i * P
        order = np.lexsort((r_i, d_i, ch_i))
        r_i, ch_i, d_i = r_i[order], ch_i[order], d_i[order]
        ldest = ch_i * P + d_i

        equad = np.zeros(NCHUNK * P, np.int64)
        equad[newpos] = qd2
        emod = np.zeros(NCHUNK * P, np.int64)
        emod[newpos] = md2

        lanestart = np.zeros(NCHUNK * P, np.int64)
        modstart = np.zeros(NCHUNK * P, np.int64)
        for c in range(NCHUNK):
            a, b = c * P, (c + 1) * P
            qs = np.zeros(P, np.int64)
            qs[1:] = np.cumsum(equad[a : b - 1])
            lanestart[a:b] = qs
            ms = np.zeros(P, np.int64)
            ms[1:] = np.cumsum(emod[a : b - 1])
            modstart[a:b] = ms

        first = np.zeros(len(r_i), bool)
        first[0] = True
        first[1:] = ldest[1:] != ldest[:-1]
        gidx = np.arange(len(r_i))
        dstart = np.zeros(len(r_i), np.int64)
        dstart[first] = gidx[first]
        dstart = np.maximum.accumulate(dstart)
        epos = gidx - dstart

        ntri = 3 * equad[ldest]
        is_tri = epos < ntri

        xg = np.zeros((ttot, P, G, D_FEAT), np.float32)
        colq = np.full((ncol, P), -1.0, np.float32)

        # full-group edges
        ce = ldest[is_tri]
        c_e = ch_i[is_tri]
        lane = lanestart[ce] + epos[is_tri] // 3
        g_e = epos[is_tri] % 3
        ti_e = tiles_arr[c_e, lane // P]
        li_e = lane % P
        xg[ti_e, li_e, g_e] = x[r_i[is_tri]]
        colq[col_arr[ti_e], li_e] = d_i[is_tri]

        # remainder edges -> triplet remainder tile, column = chunk % 3
        rr = ~is_tri
        if rr.any():
            cr = ldest[rr]
            c_r = ch_i[rr]
            rl = modstart[cr] + (epos[rr] - ntri[ldest][rr])
            assert rl.max() < P
            trip = c_r // 3
            g_r = c_r - trip * 3
            ti_r = np.array(rem_tile, np.int64)[trip]
            xg[ti_r, rl, g_r] = x[r_i[rr]]
            colq[col_arr[ti_r] + g_r, rl] = d_i[rr]

        xg_pm = np.ascontiguousarray(
            xg.transpose(1, 0, 2, 3).astype(ml_dtypes.bfloat16)
        )  # [128, ttot, G, 64]

        rc = np.zeros(NCHUNK * P, np.float32)
        rc[newpos] = recip_full[ci * SPAN : (ci + 1) * SPAN]
        recip = rc.reshape(NCHUNK, P).T
        meta = np.ascontiguousarray(
            np.concatenate([colq.T, recip], axis=1), dtype=np.float32
        )  # [128, ncol + NCHUNK]

        in_maps.append({"xg": xg_pm, "meta": meta})
    return cfg, in_maps


def _build(cfg):
    import concourse.bacc as bacc
    import concourse.mybir as mybir
    from contextlib import ExitStack

    TT, TP, ttot, ncol = cfg["TT"], cfg["TP"], cfg["ttot"], cfg["ncol"]
    tile_kind = cfg["tile_kind"]
    chunk_tiles = cfg["chunk_tiles"]
    rem_tile = cfg["rem_tile"]
    col_of_tile = cfg["col_of_tile"]
    mat_of_tile = cfg["mat_of_tile"]
    tile_of_mat = cfg["tile_of_mat"]
    segs = cfg["segs"]
    seg_of_tile = cfg["seg_of_tile"]
    nseg = len(segs)
    n_out = len(OUT_BOUNDS) - 1

    # per-chunk psum buffer and act release gate
    def buf_of(c):
        if c == NCHUNK - 1:
            return 3 * ((NTRIP) % 2)
        return 3 * ((c // 3) % 2) + c % 3

    act_gate = [0] * NCHUNK
    for c in range(NCHUNK - 1):
        act_gate[c] = rem_tile[c // 3] + 1
    act_gate[NCHUNK - 1] = ttot

    nc = bacc.Bacc()
    f32 = mybir.dt.float32
    bf16 = mybir.dt.bfloat16
    xg_ext = nc.declare_dram_parameter("xg", [P, ttot, G, D_FEAT], bf16, isOutput=False)
    meta_ext = nc.declare_dram_parameter(
        "meta", [P, ncol + NCHUNK], f32, isOutput=False
    )
    out_ext = nc.declare_dram_parameter(
        "out", [1, P, 1, NCHUNK * D_FEAT], bf16, isOutput=True
    )

    meta_sb = nc.alloc_sbuf_tensor("meta_sb", [P, ncol + NCHUNK], f32)
    colq_sb = meta_sb[:, 0:ncol]
    recip_sb = meta_sb[:, ncol : ncol + NCHUNK]
    iota_sb = nc.alloc_sbuf_tensor("iota_sb", [P, P], bf16)
    xg = nc.alloc_sbuf_tensor("xg_sb", [P, ttot, G, D_FEAT], bf16)
    ohr = nc.alloc_sbuf_tensor("ohr", [P, OHB, P], bf16)
    outst = nc.alloc_sbuf_tensor("outst", [P, NCHUNK, D_FEAT], bf16)
    ps2 = nc.alloc_psum_tensor("ps2", [P, PS_BUFS, 512], f32)
    kvidx = nc.alloc_sbuf_tensor("kvidx", [P, 1], mybir.dt.int32)

    with ExitStack() as stack:
        block = stack.enter_context(nc.Block())
        sem_x = [stack.enter_context(nc.semaphore(f"sem_x{s}")) for s in range(nseg)]
        sem_in = stack.enter_context(nc.semaphore("sem_in"))
        sem_oh = stack.enter_context(nc.semaphore("sem_oh"))
        sem_l2 = stack.enter_context(nc.semaphore("sem_l2"))
        sem_div = stack.enter_context(nc.semaphore("sem_div"))
        sem_div2 = stack.enter_context(nc.semaphore("sem_div2"))
        sem_out = stack.enter_context(nc.semaphore("sem_out"))
        sem_prep = stack.enter_context(nc.semaphore("sem_prep"))

        @block.sync
        def _(sync):
            for s, (a, b) in enumerate(segs):
                sync.dma_start(
                    out=xg[:, a:b, :], in_=xg_ext[:, a:b, :]
                ).then_inc(sem_x[s], 16)
                if s == 1:
                    sync.dma_start(out=meta_sb[:], in_=meta_ext[:]).then_inc(
                        sem_in, 16
                    )
            for gi in range(n_out - 1):
                a, b = OUT_BOUNDS[gi], OUT_BOUNDS[gi + 1]
                sync.wait_ge(sem_div, OUT_RELEASE[gi])
                sync.dma_start(
                    out=out_ext[0, :, 0, a * D_FEAT : b * D_FEAT],
                    in_=outst[:, a:b, :].rearrange("p c f -> p (c f)"),
                ).then_inc(sem_out, 16)
            sync.wait_ge(sem_out, 16 * n_out)

        @block.vector
        def _(vector):
            vector.wait_ge(sem_in, 32)
            for ti in range(ttot):
                m0, m1 = int(mat_of_tile[ti]), int(mat_of_tile[ti + 1])
                gate_m = m1 - 1 - OHB
                if gate_m >= 0:
                    vector.wait_ge(sem_l2, int(tile_of_mat[gate_m]) + 1)
                for mu in range(m0, m1):
                    cidx = col_of_tile[ti] + (mu - m0)
                    op = vector.tensor_scalar(
                        out=ohr[:, mu % OHB, :],
                        in0=iota_sb[:],
                        scalar1=colq_sb[:, cidx : cidx + 1],
                        scalar2=None,
                        op0=mybir.AluOpType.is_equal,
                    )
                    if mu == m1 - 1:
                        op.then_inc(sem_oh, 1)
            # tail-latency split: chunks 45,46 divide-by-degree on the (now
            # idle) vector engine, halving the serial activation tail
            for c in (NCHUNK - 4, NCHUNK - 3):
                vector.wait_ge(sem_l2, act_gate[c])
                vector.tensor_scalar(
                    out=outst[:, c, :],
                    in0=ps2[:, buf_of(c), 0:D_FEAT],
                    scalar1=recip_sb[:, c : c + 1],
                    scalar2=None,
                    op0=mybir.AluOpType.mult,
                ).then_inc(sem_div2, 1)

        @block.tensor
        def _(pe):
            started = set()
            for ti in range(ttot):
                s = int(seg_of_tile[ti])
                if ti == segs[s][0]:
                    pe.wait_ge(sem_x[s], 16)
                kind, val = tile_kind[ti]
                if kind == "tri":
                    c = val
                    if c not in started and (c % 3 == 0 or c == NCHUNK - 1):
                        t = c // 3
                        if t >= 2:
                            pe.wait_ge(sem_div, 3 * t - 3)
                pe.wait_ge(sem_oh, ti + 1)
                m0 = int(mat_of_tile[ti])
                if kind == "tri":
                    c = val
                    fresh = c not in started
                    started.add(c)
                    last_tri = ti == chunk_tiles[c][-1]
                    ispart = c == NCHUNK - 1
                    for g in range(G):
                        mm = pe.matmul(
                            ps2[:, buf_of(c), 0:D_FEAT],
                            lhsT=ohr[:, m0 % OHB, :],
                            rhs=xg[:, ti, g, :],
                            start=(fresh and g == 0),
                            stop=(ispart and last_tri and g == G - 1),
                        )
                        if g == G - 1:
                            mm.then_inc(sem_l2, 1)
                else:
                    t = val
                    for g in range(G):
                        c = 3 * t + g
                        mm = pe.matmul(
                            ps2[:, buf_of(c), 0:D_FEAT],
                            lhsT=ohr[:, (m0 + g) % OHB, :],
                            rhs=xg[:, ti, g, :],
                            start=False,
                            stop=True,
                        )
                        if g == G - 1:
                            mm.then_inc(sem_l2, 1)

        @block.scalar
        def _(act):
            for c in range(NCHUNK):
                if c in (NCHUNK - 4, NCHUNK - 3):
                    continue  # on DVE
                if c == 0:
                    act.wait_ge(sem_in, 32)
                act.wait_ge(sem_l2, act_gate[c])
                act.activation(
                    out=outst[:, c, :],
                    in_=ps2[:, buf_of(c), 0:D_FEAT],
                    func=mybir.ActivationFunctionType.Copy,
                    scale=recip_sb[:, c : c + 1],
                ).then_inc(sem_div, 1)

        @block.gpsimd
        def _(pool):
            pool.iota(
                iota_sb[:],
                pattern=[[1, P]],
                base=0,
                channel_multiplier=0,
                allow_small_or_imprecise_dtypes=True,
            ).then_inc(sem_in, 16)
            a, b = OUT_BOUNDS[n_out - 1], OUT_BOUNDS[n_out]
            pool.memset(kvidx[:], a * D_FEAT)
            pool.kv_writeback(
                out_ext[:],
                outst[:, a:b, :].rearrange("p (q r c) f -> p q r (c f)", q=1, r=1),
                kvidx[:],
                prepare_only=True,
                sem=sem_out,
            ).then_inc(sem_prep, 1)
            pool.wait_ge(sem_prep, 1)
            pool.wait_ge(sem_div, NCHUNK - 2)
            pool.wait_ge(sem_div2, 2)
            pool.trigger_dma(count=1)

    nc.finalize()
    return nc


def _get_built(x, edge_index):
    cfg, in_maps = _preprocess(x, edge_index)
    nc = _build(cfg)
    return cfg, in_maps, nc


def kernel(x, edge_index):
    from concourse.bass_utils import run_bass_kernel_spmd

    cfg, in_maps, nc = _get_built(np.asarray(x), np.asarray(edge_index))
    res = run_bass_kernel_spmd(nc, in_maps, core_ids=list(range(NCORES)))
    out = np.empty((N_NODES, D_FEAT), np.float32)
    for i in range(NCORES):
        dev = np.asarray(res.results[i]["out"]).astype(np.float32)
        dev = dev.reshape(P, NCHUNK, D_FEAT)
        pos_rows = dev.transpose(1, 0, 2).reshape(NCHUNK * P, D_FEAT)
        out[i * SPAN : (i + 1) * SPAN] = pos_rows[cfg["pos"][i]]
    return out


# revision 27
# speedup vs baseline: 1.0065x; 1.0065x over previous
"""Segment-mean GNN message passing (scatter-mean) on 8 TRN2 NeuronCores.

out[d] = mean over edges e with col[e]==d of x[row[e]]   (empty segments -> 0)

Design (1D destination partition per the sharding hint):
- Dests sharded across 8 cores (6250 each); edges partitioned by dest on
  host, which also gathers x[row] per edge slot (the halo exchange) into a
  bf16 tile stream. Full floor(d/3) triple groups live in 128-group tiles
  sharing one one-hot scatter matrix; each chunk-triplet's mod-3 leftover
  edges share ONE remainder tile whose three slot-columns carry three
  distinct one-hots into three different psum accumulators. Dest->chunk
  assignment is bin-packed (LPT + swap repair + mod pre-promotion) against
  a shared 5/6-tile profile, hitting the exact capacity lower bound
  (263 tiles/core, <1% padding vs raw edges).
- TensorE scatters each slot column via one-hot matmuls (cost is
  output-width-bound, so contraction passes are cheap); VectorE builds the
  one-hot matrices (is_equal vs an on-device iota) far ahead of the input
  stream; ScalarE+VectorE divide by degree writing bf16.
- One DMA per triplet (per-chunk for the last two); output DMAs write a
  packed partition-major DRAM layout (512B descriptors) and are
  release-gated so their transfers queue behind all input transfers; the
  final group goes out through a prepared SWDGE kv_writeback fired by a
  ~60ns trigger. Host converts bf16->f32 and inverts the permutation.
The DMA engine stream is fully packed: total time = first-DMA issue
latency + bytes/360GBps + completion-semaphore close-out.
"""

import sys

for _p in ("/opt/trn_rl_repo",):
    if _p not in sys.path:
        sys.path.insert(0, _p)

import numpy as np
import ml_dtypes

N_NODES = 50000
D_FEAT = 64
N_EDGES = 800000
NCORES = 8
SPAN = N_NODES // NCORES  # 6250 dests per core
P = 128
NCHUNK = (SPAN + P - 1) // P  # 49 (last chunk has 106 dests)
G = 3
NTRIP = 16  # chunks 0..47 in triplets; chunk 48 (partial) standalone
PS_BUFS = 6  # psum accumulators: triplet t uses 3*(t%2)+{0,1,2}
OHB = 24  # one-hot matrix ring depth
OUT_BOUNDS = [0, 8, 16, 24, 32, 40, 45, 49]
OUT_RELEASE = [33, 36, 39, 42, 45, 45]


def _pack_bins_v2(qd_in, md_in, ncap640, seed=0, mslack=120):
    """Pack SPAN dests into 48 main bins (128 dests each) + a partial bin
    (106): ncap640 main bins capped at 640 quads (5 tiles), rest at 768;
    every main bin's mod-sum <= 128 (remainder-column capacity), repaired
    by quad-equal swaps or promotion to ceil groups. The partial bin uses
    ceil(d/3) groups. Returns (newpos, qd, md, mainsums) or None."""
    import heapq

    nd = len(qd_in)
    qd = qd_in.copy()
    md = md_in.copy()
    npart = SPAN - (NCHUNK - 1) * P  # 106
    nmain = NCHUNK - 1  # 48
    qceil = qd + (md > 0)
    asc = np.argsort(qceil, kind="stable")
    part = asc[:npart]
    inpart = np.zeros(nd, bool)
    inpart[part] = True
    qd[part] = qceil[part]
    md[part] = 0

    main = np.where(~inpart)[0]
    if seed:
        rng = np.random.RandomState(seed)
        jitter = rng.rand(len(main))
        main = main[np.lexsort((jitter, -qd[main]))]
    else:
        main = main[np.argsort(-qd[main], kind="stable")]
    caps = np.array([640] * ncap640 + [768] * (nmain - ncap640), np.int64)
    counts = np.zeros(nmain, np.int64)
    sums = np.zeros(nmain, np.int64)
    msums = np.zeros(nmain, np.int64)
    heap = [(-int(caps[b]), b) for b in range(nmain)]
    heapq.heapify(heap)
    members = [[] for _ in range(nmain)]
    for d in main:
        while True:
            _, b = heapq.heappop(heap)
            if counts[b] < P:
                break
        members[b].append(int(d))
        counts[b] += 1
        sums[b] += qd[d]
        msums[b] += md[d]
        if counts[b] < P:
            heapq.heappush(heap, (-(int(caps[b]) - int(sums[b])), b))

    def swap(b, b2, da, db):
        members[b].remove(da)
        members[b2].remove(db)
        members[b].append(db)
        members[b2].append(da)
        sums[b] += qd[db] - qd[da]
        sums[b2] += qd[da] - qd[db]
        msums[b] += md[db] - md[da]
        msums[b2] += md[da] - md[db]

    # repair quad-cap violations
    for _ in range(4000):
        over = np.where(sums > caps)[0]
        if len(over) == 0:
            break
        b = int(over[np.argmax(sums[over] - caps[over])])
        under = np.where(caps - sums > 0)[0]
        done = False
        for b2 in under[np.argsort(-(caps[under] - sums[under]))][:16]:
            b2 = int(b2)
            for da in sorted(members[b], key=lambda d: -qd[d])[:48]:
                for db in sorted(members[b2], key=lambda d: qd[d])[:48]:
                    dq = int(qd[da] - qd[db])
                    if dq > 0 and sums[b2] + dq <= caps[b2]:
                        swap(b, b2, da, db)
                        done = True
                        break
                if done:
                    break
            if done:
                break
        if not done:
            return None
    # repair mod-cap violations (msum <= 128)
    for _ in range(4000):
        over = np.where(msums > P)[0]
        if len(over) == 0:
            break
        b = int(over[0])
        done = False
        under = np.where(msums < P)[0]
        cand_a = sorted([d for d in members[b] if md[d] > 0], key=lambda d: -md[d])[
            :32
        ]
        for b2 in under[np.argsort(msums[under])][:24]:
            b2 = int(b2)
            lowm = {}
            for d in members[b2]:
                q = int(qd[d])
                if q not in lowm or md[d] < md[lowm[q]]:
                    lowm[q] = d
            for da in cand_a:
                db = lowm.get(int(qd[da]))
                if (
                    db is not None
                    and md[db] < md[da]
                    and msums[b2] + md[da] - md[db] <= P
                ):
                    swap(b, b2, da, db)
                    done = True
                    break
            if done:
                break
        if not done:
            # promote a mod!=0 dest to ceil groups if quad room allows
            for da in cand_a:
                if sums[b] + 1 <= caps[b]:
                    sums[b] += 1
                    msums[b] -= md[da]
                    qd[da] += 1
                    md[da] = 0
                    done = True
                    break
            if not done:
                return None
    newpos = np.empty(nd, np.int64)
    for b in range(nmain):
        assert len(members[b]) == P
        newpos[np.array(members[b], np.int64)] = b * P + np.arange(P)
    newpos[part] = nmain * P + np.arange(npart)
    return newpos, qd, md, sums


def _preprocess(x, edge_index):
    x = np.ascontiguousarray(x, dtype=np.float32)
    row = edge_index[0].astype(np.int64)
    col = edge_index[1].astype(np.int64)

    deg = np.bincount(col, minlength=N_NODES).astype(np.int64)
    recip_full = (1.0 / np.maximum(deg, 1.0)).astype(np.float32)

    core = col // SPAN
    lcol = col - core * SPAN

    q_all = deg // G
    m_all = deg - q_all * G

    packs = []
    for ci in range(NCORES):
        qd = q_all[ci * SPAN : (ci + 1) * SPAN]
        md = m_all[ci * SPAN : (ci + 1) * SPAN]
        got = None
        for ncap in (46, 45, 44, 43, 42, 41, 40, 39, 38, 37, 36, 32, 24, 0):
            for seed in (0, 1, 2, 3, 4):
                got = _pack_bins_v2(qd, md, ncap, seed)
                if got is not None:
                    break
            if got is not None:
                break
        assert got is not None, "packing failed"
        packs.append(got)

    binq = np.stack([p[3] for p in packs])  # [cores, 48]
    partq = np.zeros(NCORES, np.int64)
    for ci in range(NCORES):
        newpos, qd2, md2, sums = packs[ci]
        sel = newpos >= (NCHUNK - 1) * P
        partq[ci] = qd2[sel].sum()
    TT = np.maximum(1, -(-binq.max(axis=0) // P)).astype(np.int64)  # [48]
    TP = int(max(1, -(-int(partq.max()) // P)))

    # global tile order: per triplet [A tiles | B | C | remainder], then
    # partial-chunk tiles
    tile_kind = []  # per tile: ("tri", chunk) or ("rem", triplet)
    chunk_tiles = [[] for _ in range(NCHUNK)]
    rem_tile = [0] * NTRIP
    col_of_tile = []
    ncol = 0
    for t in range(NTRIP):
        for g in range(3):
            c = 3 * t + g
            for _k in range(int(TT[c])):
                chunk_tiles[c].append(len(tile_kind))
                tile_kind.append(("tri", c))
                col_of_tile.append(ncol)
                ncol += 1
        rem_tile[t] = len(tile_kind)
        tile_kind.append(("rem", t))
        col_of_tile.append(ncol)
        ncol += 3
    for _k in range(TP):
        chunk_tiles[NCHUNK - 1].append(len(tile_kind))
        tile_kind.append(("tri", NCHUNK - 1))
        col_of_tile.append(ncol)
        ncol += 1
    ttot = len(tile_kind)

    mat_of_tile = np.zeros(ttot + 1, np.int64)
    for ti in range(ttot):
        mat_of_tile[ti + 1] = mat_of_tile[ti] + (
            3 if tile_kind[ti][0] == "rem" else 1
        )
    tile_of_mat = np.zeros(int(mat_of_tile[ttot]), np.int64)
    for ti in range(ttot):
        tile_of_mat[mat_of_tile[ti] : mat_of_tile[ti + 1]] = ti

    # DMA segments: one per triplet, except the last two triplets stream
    # per-chunk (plus their remainder tiles) so the PE/act tail pipelines
    # at fine grain; partial chunk last
    segs = []
    for t in range(NTRIP - 2):
        a = chunk_tiles[3 * t][0]
        b = rem_tile[t] + 1
        segs.append((a, b))
    for t in (NTRIP - 2, NTRIP - 1):
        for g in range(3):
            c = 3 * t + g
            a, b = chunk_tiles[c][0], chunk_tiles[c][-1] + 1
            if g == 2:
                b = rem_tile[t] + 1  # fold the 1-tile remainder in: a lone
                # 384B/partition descriptor would pay the <512B 2x penalty
            segs.append((a, b))
    segs.append((rem_tile[NTRIP - 1] + 1, ttot))
    seg_of_tile = np.zeros(ttot, np.int64)
    for s, (a, b) in enumerate(segs):
        seg_of_tile[a:b] = s

    cfg = dict(
        TT=TT,
        TP=TP,
        ttot=ttot,
        ncol=ncol,
        tile_kind=tile_kind,
        chunk_tiles=chunk_tiles,
        rem_tile=rem_tile,
        col_of_tile=col_of_tile,
        mat_of_tile=mat_of_tile,
        tile_of_mat=tile_of_mat,
        segs=segs,
        seg_of_tile=seg_of_tile,
        pos=[p[0] for p in packs],
    )

    tmax = int(TT.max())
    tiles_arr = np.zeros((NCHUNK, max(tmax, TP)), np.int64)
    for c in range(NCHUNK):
        for k, ti in enumerate(chunk_tiles[c]):
            tiles_arr[c, k] = ti
    col_arr = np.array(col_of_tile, np.int64)

    in_maps = []
    for ci in range(NCORES):
        newpos, qd2, md2, sums = packs[ci]
        m = core == ci
        r_i = row[m]
        pe_i = newpos[lcol[m]]
        ch_i = pe_i // P
        d_i = pe_i - ch_i * P
        order = np.lexsort((r_i, d_i, ch_i))
        r_i, ch_i, d_i = r_i[order], ch_i[order], d_i[order]
        ldest = ch_i * P + d_i

        equad = np.zeros(NCHUNK * P, np.int64)
        equad[newpos] = qd2
        emod = np.zeros(NCHUNK * P, np.int64)
        emod[newpos] = md2

        lanestart = np.zeros(NCHUNK * P, np.int64)
        modstart = np.zeros(NCHUNK * P, np.int64)
        for c in range(NCHUNK):
            a, b = c * P, (c + 1) * P
            qs = np.zeros(P, np.int64)
            qs[1:] = np.cumsum(equad[a : b - 1])
            lanestart[a:b] = qs
            ms = np.zeros(P, np.int64)
            ms[1:] = np.cumsum(emod[a : b - 1])
            modstart[a:b] = ms

        first = np.zeros(len(r_i), bool)
        first[0] = True
        first[1:] = ldest[1:] != ldest[:-1]
        gidx = np.arange(len(r_i))
        dstart = np.zeros(len(r_i), np.int64)
        dstart[first] = gidx[first]
        dstart = np.maximum.accumulate(dstart)
        epos = gidx - dstart

        ntri = 3 * equad[ldest]
        is_tri = epos < ntri

        xg = np.zeros((ttot, P, G, D_FEAT), np.float32)
        colq = np.full((ncol, P), -1.0, np.float32)

        # full-group edges
        ce = ldest[is_tri]
        c_e = ch_i[is_tri]
        lane = lanestart[ce] + epos[is_tri] // 3
        g_e = epos[is_tri] % 3
        ti_e = tiles_arr[c_e, lane // P]
        li_e = lane % P
        xg[ti_e, li_e, g_e] = x[r_i[is_tri]]
        colq[col_arr[ti_e], li_e] = d_i[is_tri]

        # remainder edges -> triplet remainder tile, column = chunk % 3
        rr = ~is_tri
        if rr.any():
            cr = ldest[rr]
            c_r = ch_i[rr]
            rl = modstart[cr] + (epos[rr] - ntri[ldest][rr])
            assert rl.max() < P
            trip = c_r // 3
            g_r = c_r - trip * 3
            ti_r = np.array(rem_tile, np.int64)[trip]
            xg[ti_r, rl, g_r] = x[r_i[rr]]
            colq[col_arr[ti_r] + g_r, rl] = d_i[rr]

        xg_pm = np.ascontiguousarray(
            xg.transpose(1, 0, 2, 3).astype(ml_dtypes.bfloat16)
        )  # [128, ttot, G, 64]

        rc = np.zeros(NCHUNK * P, np.float32)
        rc[newpos] = recip_full[ci * SPAN : (ci + 1) * SPAN]
        recip = rc.reshape(NCHUNK, P).T
        meta = np.ascontiguousarray(
            np.concatenate([colq.T, recip], axis=1), dtype=np.float32
        )  # [128, ncol + NCHUNK]

        in_maps.append({"xg": xg_pm, "meta": meta})
    return cfg, in_maps


def _build(cfg):
    import concourse.bacc as bacc
    import concourse.mybir as mybir
    from contextlib import ExitStack

    TT, TP, ttot, ncol = cfg["TT"], cfg["TP"], cfg["ttot"], cfg["ncol"]
    tile_kind = cfg["tile_kind"]
    chunk_tiles = cfg["chunk_tiles"]
    rem_tile = cfg["rem_tile"]
    col_of_tile = cfg["col_of_tile"]
    mat_of_tile = cfg["mat_of_tile"]
    tile_of_mat = cfg["tile_of_mat"]
    segs = cfg["segs"]
    seg_of_tile = cfg["seg_of_tile"]
    nseg = len(segs)
    n_out = len(OUT_BOUNDS) - 1

    # per-chunk psum buffer and act release gate
    def buf_of(c):
        if c == NCHUNK - 1:
            return 3 * ((NTRIP) % 2)
        return 3 * ((c // 3) % 2) + c % 3

    act_gate = [0] * NCHUNK
    for c in range(NCHUNK - 1):
        act_gate[c] = rem_tile[c // 3] + 1
    act_gate[NCHUNK - 1] = ttot

    nc = bacc.Bacc()
    f32 = mybir.dt.float32
    bf16 = mybir.dt.bfloat16
    xg_ext = nc.declare_dram_parameter("xg", [P, ttot, G, D_FEAT], bf16, isOutput=False)
    meta_ext = nc.declare_dram_parameter(
        "meta", [P, ncol + NCHUNK], f32, isOutput=False
    )
    out_ext = nc.declare_dram_parameter(
        "out", [1, P, 1, NCHUNK * D_FEAT], bf16, isOutput=True
    )

    meta_sb = nc.alloc_sbuf_tensor("meta_sb", [P, ncol + NCHUNK], f32)
    colq_sb = meta_sb[:, 0:ncol]
    recip_sb = meta_sb[:, ncol : ncol + NCHUNK]
    iota_sb = nc.alloc_sbuf_tensor("iota_sb", [P, P], bf16)
    xg = nc.alloc_sbuf_tensor("xg_sb", [P, ttot, G, D_FEAT], bf16)
    ohr = nc.alloc_sbuf_tensor("ohr", [P, OHB, P], bf16)
    outst = nc.alloc_sbuf_tensor("outst", [P, NCHUNK, D_FEAT], bf16)
    ps2 = nc.alloc_psum_tensor("ps2", [P, PS_BUFS, 512], f32)
    kvidx = nc.alloc_sbuf_tensor("kvidx", [P, 1], mybir.dt.int32)

    with ExitStack() as stack:
        block = stack.enter_context(nc.Block())
        sem_x = [stack.enter_context(nc.semaphore(f"sem_x{s}")) for s in range(nseg)]
        sem_in = stack.enter_context(nc.semaphore("sem_in"))
        sem_oh = stack.enter_context(nc.semaphore("sem_oh"))
        sem_l2 = stack.enter_context(nc.semaphore("sem_l2"))
        sem_div = stack.enter_context(nc.semaphore("sem_div"))
        sem_div2 = stack.enter_context(nc.semaphore("sem_div2"))
        sem_out = stack.enter_context(nc.semaphore("sem_out"))
        sem_prep = stack.enter_context(nc.semaphore("sem_prep"))

        @block.sync
        def _(sync):
            for s, (a, b) in enumerate(segs):
                sync.dma_start(
                    out=xg[:, a:b, :], in_=xg_ext[:, a:b, :]
                ).then_inc(sem_x[s], 16)
                if s == 1:
                    sync.dma_start(out=meta_sb[:], in_=meta_ext[:]).then_inc(
                        sem_in, 16
                    )
            for gi in range(n_out - 1):
                a, b = OUT_BOUNDS[gi], OUT_BOUNDS[gi + 1]
                sync.wait_ge(sem_div, OUT_RELEASE[gi])
                sync.dma_start(
                    out=out_ext[0, :, 0, a * D_FEAT : b * D_FEAT],
                    in_=outst[:, a:b, :].rearrange("p c f -> p (c f)"),
                ).then_inc(sem_out, 16)
            sync.wait_ge(sem_out, 16 * n_out)

        @block.vector
        def _(vector):
            vector.wait_ge(sem_in, 32)
            for ti in range(ttot):
                m0, m1 = int(mat_of_tile[ti]), int(mat_of_tile[ti + 1])
                gate_m = m1 - 1 - OHB
                if gate_m >= 0:
                    vector.wait_ge(sem_l2, int(tile_of_mat[gate_m]) + 1)
                for mu in range(m0, m1):
                    cidx = col_of_tile[ti] + (mu - m0)
                    op = vector.tensor_scalar(
                        out=ohr[:, mu % OHB, :],
                        in0=iota_sb[:],
                        scalar1=colq_sb[:, cidx : cidx + 1],
                        scalar2=None,
                        op0=mybir.AluOpType.is_equal,
                    )
                    if mu == m1 - 1:
                        op.then_inc(sem_oh, 1)
            # tail-latency split: chunks 45,46 divide-by-degree on the (now
            # idle) vector engine, halving the serial activation tail
            for c in (NCHUNK - 4, NCHUNK - 3):
                vector.wait_ge(sem_l2, act_gate[c])
                vector.tensor_scalar(
                    out=outst[:, c, :],
                    in0=ps2[:, buf_of(c), 0:D_FEAT],
                    scalar1=recip_sb[:, c : c + 1],
                    scalar2=None,
                    op0=mybir.AluOpType.mult,
                ).then_inc(sem_div2, 1)

        @block.tensor
        def _(pe):
            started = set()
            for ti in range(ttot):
                s = int(seg_of_tile[ti])
                if ti == segs[s][0]:
                    pe.wait_ge(sem_x[s], 16)
                kind, val = tile_kind[ti]
                if kind == "tri":
                    c = val
                    if c not in started and (c % 3 == 0 or c == NCHUNK - 1):
                        t = c // 3
                        if t >= 2:
                            pe.wait_ge(sem_div, 3 * t - 3)
                pe.wait_ge(sem_oh, ti + 1)
                m0 = int(mat_of_tile[ti])
                if kind == "tri":
                    c = val
                    fresh = c not in started
                    started.add(c)
                    last_tri = ti == chunk_tiles[c][-1]
                    ispart = c == NCHUNK - 1
                    for g in range(G):
                        mm = pe.matmul(
                            ps2[:, buf_of(c), 0:D_FEAT],
                            lhsT=ohr[:, m0 % OHB, :],
                            rhs=xg[:, ti, g, :],
                            start=(fresh and g == 0),
                            stop=(ispart and last_tri and g == G - 1),
                        )
                        if g == G - 1:
                            mm.then_inc(sem_l2, 1)
                else:
                    t = val
                    for g in range(G):
                        c = 3 * t + g
                        mm = pe.matmul(
                            ps2[:, buf_of(c), 0:D_FEAT],
                            lhsT=ohr[:, (m0 + g) % OHB, :],
                            rhs=xg[:, ti, g, :],
                            start=False,
                            stop=True,
                        )
                        if g == G - 1:
                            mm.then_inc(sem_l2, 1)

        @block.scalar
        def _(act):
            for c in range(NCHUNK):
                if c in (NCHUNK - 4, NCHUNK - 3):
                    continue  # on DVE
                if c == 0:
                    act.wait_ge(sem_in, 32)
                act.wait_ge(sem_l2, act_gate[c])
                act.activation(
                    out=outst[:, c, :],
                    in_=ps2[:, buf_of(c), 0:D_FEAT],
                    func=mybir.ActivationFunctionType.Copy,
                    scale=recip_sb[:, c : c + 1],
                ).then_inc(sem_div, 1)

        @block.gpsimd
        def _(pool):
            pool.iota(
                iota_sb[:],
                pattern=[[1, P]],
                base=0,
                channel_multiplier=0,
                allow_small_or_imprecise_dtypes=True,
            ).then_inc(sem_in, 16)
            a, b = OUT_BOUNDS[n_out - 1], OUT_BOUNDS[n_out]
            pool.memset(kvidx[:], a * D_FEAT)
            pool.kv_writeback(
                out_ext[:],
                outst[:, a:b, :].rearrange("p (q r c) f -> p q r (c f)", q=1, r=1),
                kvidx[:],
                prepare_only=True,
                sem=sem_out,
            ).then_inc(sem_prep, 1)
            pool.wait_ge(sem_prep, 1)
            pool.wait_ge(sem_div, NCHUNK - 2)
            pool.wait_ge(sem_div2, 2)
            pool.trigger_dma(count=1)

    nc.finalize()
    return nc


def _get_built(x, edge_index):
    cfg, in_maps = _preprocess(x, edge_index)
    nc = _build(cfg)
    return cfg, in_maps, nc


def kernel(x, edge_index):
    from concourse.bass_utils import run_bass_kernel_spmd

    cfg, in_maps, nc = _get_built(np.asarray(x), np.asarray(edge_index))
    res = run_bass_kernel_spmd(nc, in_maps, core_ids=list(range(NCORES)))
    out = np.empty((N_NODES, D_FEAT), np.float32)
    for i in range(NCORES):
        dev = np.asarray(res.results[i]["out"]).astype(np.float32)
        dev = dev.reshape(P, NCHUNK, D_FEAT)
        pos_rows = dev.transpose(1, 0, 2).reshape(NCHUNK * P, D_FEAT)
        out[i * SPAN : (i + 1) * SPAN] = pos_rows[cfg["pos"][i]]
    return out


# revision 28
# speedup vs baseline: 1.0099x; 1.0034x over previous
"""Segment-mean GNN message passing (scatter-mean) on 8 TRN2 NeuronCores.

out[d] = mean over edges e with col[e]==d of x[row[e]]   (empty segments -> 0)

Design (1D destination partition per the sharding hint):
- Dests sharded across 8 cores (6250 each); edges partitioned by dest on
  host, which also gathers x[row] per edge slot (the halo exchange) into a
  bf16 tile stream. Full floor(d/3) triple groups live in 128-group tiles
  sharing one one-hot scatter matrix; each chunk-triplet's mod-3 leftover
  edges share ONE remainder tile whose three slot-columns carry three
  distinct one-hots into three different psum accumulators. Dest->chunk
  assignment is bin-packed (LPT + swap repair + mod pre-promotion) against
  a shared 5/6-tile profile, hitting the exact capacity lower bound
  (263 tiles/core, <1% padding vs raw edges).
- TensorE scatters each slot column via one-hot matmuls (cost is
  output-width-bound, so contraction passes are cheap); VectorE builds the
  one-hot matrices (is_equal vs an on-device iota) far ahead of the input
  stream; ScalarE+VectorE divide by degree writing bf16.
- One DMA per triplet (per-chunk for the last two); output DMAs write a
  packed partition-major DRAM layout (512B descriptors) and are
  release-gated so their transfers queue behind all input transfers; the
  final group goes out through a prepared SWDGE kv_writeback fired by a
  ~60ns trigger. Host converts bf16->f32 and inverts the permutation.
The DMA engine stream is fully packed: total time = first-DMA issue
latency + bytes/360GBps + completion-semaphore close-out.
"""

import sys

for _p in ("/opt/trn_rl_repo",):
    if _p not in sys.path:
        sys.path.insert(0, _p)

import numpy as np
import ml_dtypes

N_NODES = 50000
D_FEAT = 64
N_EDGES = 800000
NCORES = 8
SPAN = N_NODES // NCORES  # 6250 dests per core
P = 128
NCHUNK = (SPAN + P - 1) // P  # 49 (last chunk has 106 dests)
G = 3
NTRIP = 16  # chunks 0..47 in triplets; chunk 48 (partial) standalone
PS_BUFS = 6  # psum accumulators: triplet t uses 3*(t%2)+{0,1,2}
OHB = 24  # one-hot matrix ring depth
OUT_BOUNDS = [0, 8, 16, 24, 32, 40, 45, 49]
OUT_RELEASE = [33, 36, 39, 42, 45, 45]


def _pack_bins_v2(qd_in, md_in, ncap640, seed=0, mslack=120):
    """Pack SPAN dests into 48 main bins (128 dests each) + a partial bin
    (106): ncap640 main bins capped at 640 quads (5 tiles), rest at 768;
    every main bin's mod-sum <= 128 (remainder-column capacity), repaired
    by quad-equal swaps or promotion to ceil groups. The partial bin uses
    ceil(d/3) groups. Returns (newpos, qd, md, mainsums) or None."""
    import heapq

    nd = len(qd_in)
    qd = qd_in.copy()
    md = md_in.copy()
    npart = SPAN - (NCHUNK - 1) * P  # 106
    nmain = NCHUNK - 1  # 48
    qceil = qd + (md > 0)
    asc = np.argsort(qceil, kind="stable")
    part = asc[:npart]
    inpart = np.zeros(nd, bool)
    inpart[part] = True
    qd[part] = qceil[part]
    md[part] = 0

    main = np.where(~inpart)[0]
    if seed:
        rng = np.random.RandomState(seed)
        jitter = rng.rand(len(main))
        main = main[np.lexsort((jitter, -qd[main]))]
    else:
        main = main[np.argsort(-qd[main], kind="stable")]
    caps = np.array([640] * ncap640 + [768] * (nmain - ncap640), np.int64)
    counts = np.zeros(nmain, np.int64)
    sums = np.zeros(nmain, np.int64)
    msums = np.zeros(nmain, np.int64)
    heap = [(-int(caps[b]), b) for b in range(nmain)]
    heapq.heapify(heap)
    members = [[] for _ in range(nmain)]
    for d in main:
        while True:
            _, b = heapq.heappop(heap)
            if counts[b] < P:
                break
        members[b].append(int(d))
        counts[b] += 1
        sums[b] += qd[d]
        msums[b] += md[d]
        if counts[b] < P:
            heapq.heappush(heap, (-(int(caps[b]) - int(sums[b])), b))

    def swap(b, b2, da, db):
        members[b].remove(da)
        members[b2].remove(db)
        members[b].append(db)
        members[b2].append(da)
        sums[b] += qd[db] - qd[da]
        sums[b2] += qd[da] - qd[db]
        msums[b] += md[db] - md[da]
        msums[b2] += md[da] - md[db]

    # repair quad-cap violations
    for _ in range(4000):
        over = np.where(sums > caps)[0]
        if len(over) == 0:
            break
        b = int(over[np.argmax(sums[over] - caps[over])])
        under = np.where(caps - sums > 0)[0]
        done = False
        for b2 in under[np.argsort(-(caps[under] - sums[under]))][:16]:
            b2 = int(b2)
            for da in sorted(members[b], key=lambda d: -qd[d])[:48]:
                for db in sorted(members[b2], key=lambda d: qd[d])[:48]:
                    dq = int(qd[da] - qd[db])
                    if dq > 0 and sums[b2] + dq <= caps[b2]:
                        swap(b, b2, da, db)
                        done = True
                        break
                if done:
                    break
            if done:
                break
        if not done:
            return None
    # repair mod-cap violations (msum <= 128)
    for _ in range(4000):
        over = np.where(msums > P)[0]
        if len(over) == 0:
            break
        b = int(over[0])
        done = False
        under = np.where(msums < P)[0]
        cand_a = sorted([d for d in members[b] if md[d] > 0], key=lambda d: -md[d])[
            :32
        ]
        for b2 in under[np.argsort(msums[under])][:24]:
            b2 = int(b2)
            lowm = {}
            for d in members[b2]:
                q = int(qd[d])
                if q not in lowm or md[d] < md[lowm[q]]:
                    lowm[q] = d
            for da in cand_a:
                db = lowm.get(int(qd[da]))
                if (
                    db is not None
                    and md[db] < md[da]
                    and msums[b2] + md[da] - md[db] <= P
                ):
                    swap(b, b2, da, db)
                    done = True
                    break
            if done:
                break
        if not done:
            # promote a mod!=0 dest to ceil groups if quad room allows
            for da in cand_a:
                if sums[b] + 1 <= caps[b]:
                    sums[b] += 1
                    msums[b] -= md[da]
                    qd[da] += 1
                    md[da] = 0
                    done = True
                    break
            if not done:
                return None
    newpos = np.empty(nd, np.int64)
    for b in range(nmain):
        assert len(members[b]) == P
        newpos[np.array(members[b], np.int64)] = b * P + np.arange(P)
    newpos[part] = nmain * P + np.arange(npart)
    return newpos, qd, md, sums


def _preprocess(x, edge_index):
    x = np.ascontiguousarray(x, dtype=np.float32)
    row = edge_index[0].astype(np.int64)
    col = edge_index[1].astype(np.int64)

    deg = np.bincount(col, minlength=N_NODES).astype(np.int64)
    recip_full = (1.0 / np.maximum(deg, 1.0)).astype(np.float32)

    core = col // SPAN
    lcol = col - core * SPAN

    q_all = deg // G
    m_all = deg - q_all * G

    packs = []
    for ci in range(NCORES):
        qd = q_all[ci * SPAN : (ci + 1) * SPAN]
        md = m_all[ci * SPAN : (ci + 1) * SPAN]
        got = None
        for ncap in (46, 45, 44, 43, 42, 41, 40, 39, 38, 37, 36, 32, 24, 0):
            for seed in (0, 1, 2, 3, 4):
                got = _pack_bins_v2(qd, md, ncap, seed)
                if got is not None:
                    break
            if got is not None:
                break
        assert got is not None, "packing failed"
        packs.append(got)

    binq = np.stack([p[3] for p in packs])  # [cores, 48]
    partq = np.zeros(NCORES, np.int64)
    for ci in range(NCORES):
        newpos, qd2, md2, sums = packs[ci]
        sel = newpos >= (NCHUNK - 1) * P
        partq[ci] = qd2[sel].sum()
    TT = np.maximum(1, -(-binq.max(axis=0) // P)).astype(np.int64)  # [48]
    TP = int(max(1, -(-int(partq.max()) // P)))

    # global tile order: per triplet [A tiles | B | C | remainder], then
    # partial-chunk tiles
    tile_kind = []  # per tile: ("tri", chunk) or ("rem", triplet)
    chunk_tiles = [[] for _ in range(NCHUNK)]
    rem_tile = [0] * NTRIP
    col_of_tile = []
    ncol = 0
    for t in range(NTRIP):
        for g in range(3):
            c = 3 * t + g
            for _k in range(int(TT[c])):
                chunk_tiles[c].append(len(tile_kind))
                tile_kind.append(("tri", c))
                col_of_tile.append(ncol)
                ncol += 1
        rem_tile[t] = len(tile_kind)
        tile_kind.append(("rem", t))
        col_of_tile.append(ncol)
        ncol += 3
    for _k in range(TP):
        chunk_tiles[NCHUNK - 1].append(len(tile_kind))
        tile_kind.append(("tri", NCHUNK - 1))
        col_of_tile.append(ncol)
        ncol += 1
    ttot = len(tile_kind)

    mat_of_tile = np.zeros(ttot + 1, np.int64)
    for ti in range(ttot):
        mat_of_tile[ti + 1] = mat_of_tile[ti] + (
            3 if tile_kind[ti][0] == "rem" else 1
        )
    tile_of_mat = np.zeros(int(mat_of_tile[ttot]), np.int64)
    for ti in range(ttot):
        tile_of_mat[mat_of_tile[ti] : mat_of_tile[ti + 1]] = ti

    # DMA segments: one per triplet, except the last two triplets stream
    # per-chunk (plus their remainder tiles) so the PE/act tail pipelines
    # at fine grain; partial chunk last
    segs = []
    for t in range(NTRIP - 2):
        a = chunk_tiles[3 * t][0]
        b = rem_tile[t] + 1
        segs.append((a, b))
    for t in (NTRIP - 2, NTRIP - 1):
        for g in range(3):
            c = 3 * t + g
            a, b = chunk_tiles[c][0], chunk_tiles[c][-1] + 1
            if g == 2:
                b = rem_tile[t] + 1  # fold the 1-tile remainder in: a lone
                # 384B/partition descriptor would pay the <512B 2x penalty
            segs.append((a, b))
    segs.append((rem_tile[NTRIP - 1] + 1, ttot))
    seg_of_tile = np.zeros(ttot, np.int64)
    for s, (a, b) in enumerate(segs):
        seg_of_tile[a:b] = s

    cfg = dict(
        TT=TT,
        TP=TP,
        ttot=ttot,
        ncol=ncol,
        tile_kind=tile_kind,
        chunk_tiles=chunk_tiles,
        rem_tile=rem_tile,
        col_of_tile=col_of_tile,
        mat_of_tile=mat_of_tile,
        tile_of_mat=tile_of_mat,
        segs=segs,
        seg_of_tile=seg_of_tile,
        pos=[p[0] for p in packs],
    )

    tmax = int(TT.max())
    tiles_arr = np.zeros((NCHUNK, max(tmax, TP)), np.int64)
    for c in range(NCHUNK):
        for k, ti in enumerate(chunk_tiles[c]):
            tiles_arr[c, k] = ti
    col_arr = np.array(col_of_tile, np.int64)

    in_maps = []
    for ci in range(NCORES):
        newpos, qd2, md2, sums = packs[ci]
        m = core == ci
        r_i = row[m]
        pe_i = newpos[lcol[m]]
        ch_i = pe_i // P
        d_i = pe_i - ch_i * P
        order = np.lexsort((r_i, d_i, ch_i))
        r_i, ch_i, d_i = r_i[order], ch_i[order], d_i[order]
        ldest = ch_i * P + d_i

        equad = np.zeros(NCHUNK * P, np.int64)
        equad[newpos] = qd2
        emod = np.zeros(NCHUNK * P, np.int64)
        emod[newpos] = md2

        lanestart = np.zeros(NCHUNK * P, np.int64)
        modstart = np.zeros(NCHUNK * P, np.int64)
        for c in range(NCHUNK):
            a, b = c * P, (c + 1) * P
            qs = np.zeros(P, np.int64)
            qs[1:] = np.cumsum(equad[a : b - 1])
            lanestart[a:b] = qs
            ms = np.zeros(P, np.int64)
            ms[1:] = np.cumsum(emod[a : b - 1])
            modstart[a:b] = ms

        first = np.zeros(len(r_i), bool)
        first[0] = True
        first[1:] = ldest[1:] != ldest[:-1]
        gidx = np.arange(len(r_i))
        dstart = np.zeros(len(r_i), np.int64)
        dstart[first] = gidx[first]
        dstart = np.maximum.accumulate(dstart)
        epos = gidx - dstart

        ntri = 3 * equad[ldest]
        is_tri = epos < ntri

        xg = np.zeros((ttot, P, G, D_FEAT), np.float32)
        colq = np.full((ncol, P), -1.0, np.float32)

        # full-group edges
        ce = ldest[is_tri]
        c_e = ch_i[is_tri]
        lane = lanestart[ce] + epos[is_tri] // 3
        g_e = epos[is_tri] % 3
        ti_e = tiles_arr[c_e, lane // P]
        li_e = lane % P
        xg[ti_e, li_e, g_e] = x[r_i[is_tri]]
        colq[col_arr[ti_e], li_e] = d_i[is_tri]

        # remainder edges -> triplet remainder tile, column = chunk % 3
        rr = ~is_tri
        if rr.any():
            cr = ldest[rr]
            c_r = ch_i[rr]
            rl = modstart[cr] + (epos[rr] - ntri[ldest][rr])
            assert rl.max() < P
            trip = c_r // 3
            g_r = c_r - trip * 3
            ti_r = np.array(rem_tile, np.int64)[trip]
            xg[ti_r, rl, g_r] = x[r_i[rr]]
            colq[col_arr[ti_r] + g_r, rl] = d_i[rr]

        xg_pm = np.ascontiguousarray(
            xg.transpose(1, 0, 2, 3).astype(ml_dtypes.bfloat16)
        )  # [128, ttot, G, 64]

        rc = np.zeros(NCHUNK * P, np.float32)
        rc[newpos] = recip_full[ci * SPAN : (ci + 1) * SPAN]
        recip = np.ascontiguousarray(rc.reshape(NCHUNK, P).T)
        # colq values are small ints (-1..127): exact in bf16; DVE
        # up-converts once on device (is_equal's scalar port needs f32)
        colqb = np.ascontiguousarray(colq.T.astype(ml_dtypes.bfloat16))
        in_maps.append({"xg": xg_pm, "colqb": colqb, "recip": recip})
    return cfg, in_maps


def _build(cfg):
    import concourse.bacc as bacc
    import concourse.mybir as mybir
    from contextlib import ExitStack

    TT, TP, ttot, ncol = cfg["TT"], cfg["TP"], cfg["ttot"], cfg["ncol"]
    tile_kind = cfg["tile_kind"]
    chunk_tiles = cfg["chunk_tiles"]
    rem_tile = cfg["rem_tile"]
    col_of_tile = cfg["col_of_tile"]
    mat_of_tile = cfg["mat_of_tile"]
    tile_of_mat = cfg["tile_of_mat"]
    segs = cfg["segs"]
    seg_of_tile = cfg["seg_of_tile"]
    nseg = len(segs)
    n_out = len(OUT_BOUNDS) - 1

    # per-chunk psum buffer and act release gate
    def buf_of(c):
        if c == NCHUNK - 1:
            return 3 * ((NTRIP) % 2)
        return 3 * ((c // 3) % 2) + c % 3

    act_gate = [0] * NCHUNK
    for c in range(NCHUNK - 1):
        act_gate[c] = rem_tile[c // 3] + 1
    act_gate[NCHUNK - 1] = ttot

    nc = bacc.Bacc()
    f32 = mybir.dt.float32
    bf16 = mybir.dt.bfloat16
    xg_ext = nc.declare_dram_parameter("xg", [P, ttot, G, D_FEAT], bf16, isOutput=False)
    colqb_ext = nc.declare_dram_parameter("colqb", [P, ncol], bf16, isOutput=False)
    recip_ext = nc.declare_dram_parameter("recip", [P, NCHUNK], f32, isOutput=False)
    out_ext = nc.declare_dram_parameter(
        "out", [1, P, 1, NCHUNK * D_FEAT], bf16, isOutput=True
    )

    colqb_sb = nc.alloc_sbuf_tensor("colqb_sb", [P, ncol], bf16)
    colq_sb = nc.alloc_sbuf_tensor("colq_sb", [P, ncol], f32)
    recip_sb = nc.alloc_sbuf_tensor("recip_sb", [P, NCHUNK], f32)
    iota_sb = nc.alloc_sbuf_tensor("iota_sb", [P, P], bf16)
    xg = nc.alloc_sbuf_tensor("xg_sb", [P, ttot, G, D_FEAT], bf16)
    ohr = nc.alloc_sbuf_tensor("ohr", [P, OHB, P], bf16)
    outst = nc.alloc_sbuf_tensor("outst", [P, NCHUNK, D_FEAT], bf16)
    ps2 = nc.alloc_psum_tensor("ps2", [P, PS_BUFS, 512], f32)
    kvidx = nc.alloc_sbuf_tensor("kvidx", [P, 1], mybir.dt.int32)

    with ExitStack() as stack:
        block = stack.enter_context(nc.Block())
        sem_x = [stack.enter_context(nc.semaphore(f"sem_x{s}")) for s in range(nseg)]
        sem_in = stack.enter_context(nc.semaphore("sem_in"))
        sem_oh = stack.enter_context(nc.semaphore("sem_oh"))
        sem_l2 = stack.enter_context(nc.semaphore("sem_l2"))
        sem_div = stack.enter_context(nc.semaphore("sem_div"))
        sem_div2 = stack.enter_context(nc.semaphore("sem_div2"))
        sem_out = stack.enter_context(nc.semaphore("sem_out"))
        sem_prep = stack.enter_context(nc.semaphore("sem_prep"))

        @block.sync
        def _(sync):
            for s, (a, b) in enumerate(segs):
                sync.dma_start(
                    out=xg[:, a:b, :], in_=xg_ext[:, a:b, :]
                ).then_inc(sem_x[s], 16)
                if s == 1:
                    sync.dma_start(out=colqb_sb[:], in_=colqb_ext[:]).then_inc(
                        sem_in, 16
                    )
                    sync.dma_start(out=recip_sb[:], in_=recip_ext[:]).then_inc(
                        sem_in, 16
                    )
            for gi in range(n_out - 1):
                a, b = OUT_BOUNDS[gi], OUT_BOUNDS[gi + 1]
                sync.wait_ge(sem_div, OUT_RELEASE[gi])
                sync.dma_start(
                    out=out_ext[0, :, 0, a * D_FEAT : b * D_FEAT],
                    in_=outst[:, a:b, :].rearrange("p c f -> p (c f)"),
                ).then_inc(sem_out, 16)
            sync.wait_ge(sem_out, 16 * n_out)

        @block.vector
        def _(vector):
            vector.wait_ge(sem_in, 48)
            vector.tensor_scalar(
                out=colq_sb[:],
                in0=colqb_sb[:],
                scalar1=0.0,
                scalar2=None,
                op0=mybir.AluOpType.add,
            )
            for ti in range(ttot):
                m0, m1 = int(mat_of_tile[ti]), int(mat_of_tile[ti + 1])
                gate_m = m1 - 1 - OHB
                if gate_m >= 0:
                    vector.wait_ge(sem_l2, int(tile_of_mat[gate_m]) + 1)
                for mu in range(m0, m1):
                    cidx = col_of_tile[ti] + (mu - m0)
                    op = vector.tensor_scalar(
                        out=ohr[:, mu % OHB, :],
                        in0=iota_sb[:],
                        scalar1=colq_sb[:, cidx : cidx + 1],
                        scalar2=None,
                        op0=mybir.AluOpType.is_equal,
                    )
                    if mu == m1 - 1:
                        op.then_inc(sem_oh, 1)
            # tail-latency split: chunks 45,46 divide-by-degree on the (now
            # idle) vector engine, halving the serial activation tail
            for c in (NCHUNK - 4, NCHUNK - 3):
                vector.wait_ge(sem_l2, act_gate[c])
                vector.tensor_scalar(
                    out=outst[:, c, :],
                    in0=ps2[:, buf_of(c), 0:D_FEAT],
                    scalar1=recip_sb[:, c : c + 1],
                    scalar2=None,
                    op0=mybir.AluOpType.mult,
                ).then_inc(sem_div2, 1)

        @block.tensor
        def _(pe):
            started = set()
            for ti in range(ttot):
                s = int(seg_of_tile[ti])
                if ti == segs[s][0]:
                    pe.wait_ge(sem_x[s], 16)
                kind, val = tile_kind[ti]
                if kind == "tri":
                    c = val
                    if c not in started and (c % 3 == 0 or c == NCHUNK - 1):
                        t = c // 3
                        if t >= 2:
                            pe.wait_ge(sem_div, 3 * t - 3)
                pe.wait_ge(sem_oh, ti + 1)
                m0 = int(mat_of_tile[ti])
                if kind == "tri":
                    c = val
                    fresh = c not in started
                    started.add(c)
                    last_tri = ti == chunk_tiles[c][-1]
                    ispart = c == NCHUNK - 1
                    for g in range(G):
                        mm = pe.matmul(
                            ps2[:, buf_of(c), 0:D_FEAT],
                            lhsT=ohr[:, m0 % OHB, :],
                            rhs=xg[:, ti, g, :],
                            start=(fresh and g == 0),
                            stop=(ispart and last_tri and g == G - 1),
                        )
                        if g == G - 1:
                            mm.then_inc(sem_l2, 1)
                else:
                    t = val
                    for g in range(G):
                        c = 3 * t + g
                        mm = pe.matmul(
                            ps2[:, buf_of(c), 0:D_FEAT],
                            lhsT=ohr[:, (m0 + g) % OHB, :],
                            rhs=xg[:, ti, g, :],
                            start=False,
                            stop=True,
                        )
                        if g == G - 1:
                            mm.then_inc(sem_l2, 1)

        @block.scalar
        def _(act):
            for c in range(NCHUNK):
                if c in (NCHUNK - 4, NCHUNK - 3):
                    continue  # on DVE
                if c == 0:
                    act.wait_ge(sem_in, 48)
                act.wait_ge(sem_l2, act_gate[c])
                act.activation(
                    out=outst[:, c, :],
                    in_=ps2[:, buf_of(c), 0:D_FEAT],
                    func=mybir.ActivationFunctionType.Copy,
                    scale=recip_sb[:, c : c + 1],
                ).then_inc(sem_div, 1)

        @block.gpsimd
        def _(pool):
            pool.iota(
                iota_sb[:],
                pattern=[[1, P]],
                base=0,
                channel_multiplier=0,
                allow_small_or_imprecise_dtypes=True,
            ).then_inc(sem_in, 16)
            a, b = OUT_BOUNDS[n_out - 1], OUT_BOUNDS[n_out]
            pool.memset(kvidx[:], a * D_FEAT)
            pool.kv_writeback(
                out_ext[:],
                outst[:, a:b, :].rearrange("p (q r c) f -> p q r (c f)", q=1, r=1),
                kvidx[:],
                prepare_only=True,
                sem=sem_out,
            ).then_inc(sem_prep, 1)
            pool.wait_ge(sem_prep, 1)
            pool.wait_ge(sem_div, NCHUNK - 2)
            pool.wait_ge(sem_div2, 2)
            pool.trigger_dma(count=1)

    nc.finalize()
    return nc


def _get_built(x, edge_index):
    cfg, in_maps = _preprocess(x, edge_index)
    nc = _build(cfg)
    return cfg, in_maps, nc


def kernel(x, edge_index):
    from concourse.bass_utils import run_bass_kernel_spmd

    cfg, in_maps, nc = _get_built(np.asarray(x), np.asarray(edge_index))
    res = run_bass_kernel_spmd(nc, in_maps, core_ids=list(range(NCORES)))
    out = np.empty((N_NODES, D_FEAT), np.float32)
    for i in range(NCORES):
        dev = np.asarray(res.results[i]["out"]).astype(np.float32)
        dev = dev.reshape(P, NCHUNK, D_FEAT)
        pos_rows = dev.transpose(1, 0, 2).reshape(NCHUNK * P, D_FEAT)
        out[i * SPAN : (i + 1) * SPAN] = pos_rows[cfg["pos"][i]]
    return out
